# revision 36
# baseline (speedup 1.0000x reference)
"""Trainium2 Bass kernel for the dense MoE layer (nn_MoELayer_74371653698164).

Reference computation (fp32):
    gate  = softmax(x @ Wg + bg)                    # [N, E]
    out   = sum_e gate[:, e] * (x @ We[e] + be[e])  # [N, D_OUT]

Strategy:
  - Data-parallel over tokens: each of the 8 cores gets N/8 = 1024 tokens and
    the full expert/gate weights. No collectives.
  - x is pre-transposed on the host (a weights-style layout change), so the
    contraction dim lands on partitions with a single DMA and no on-device
    transpose pass.
  - Softmax is factored: out = r * (sum_e exp_e * (x @ We[e] + be[e])) with
    r = 1 / sum_e exp_e. Logits are computed TRANSPOSED ([E, tok], E on
    partitions) so the bias add is a per-partition scalar op and exp is one
    activation over all tokens; logits here are ~N(0,1) so max-subtraction
    is unnecessary in fp32.
  - Per expert: stream We[e] from HBM, run 128x512 matmuls into a 2-bank
    [128,1024] PSUM pair, and fold the gate in with one DVE FMA per
    (expert, token-tile): acc = psum * exp[:, e] + acc. The bias term
    (exp @ be, one K=8 matmul pair per token tile) initializes acc.
  - Matmul operands are float32r: full fp32 data that the PE streams at
    1 cycle/row for N>=256 (vs 4 for strict fp32), near-fp32 precision.
  - Instruction count is minimized throughout (batched drains, paired PSUM
    banks, single big DMAs) — both dispatch overhead and real-HW sync cost
    scale with it.

kernel(**inputs) takes the FULL unsharded inputs and returns the FULL output.
"""
import os
from contextlib import ExitStack

import numpy as np

import bass_rust
import concourse.bass as bass
import concourse.mybir as mybir
import concourse.tile as tile
from concourse.bass_utils import run_bass_kernel_spmd
from concourse.masks import make_identity
from concourse.vector_clock import ScopedClock

# Problem shape (hardcoded per harness contract).
N_TOKENS, D_IN, D_OUT, E = 8192, 1024, 1024, 8
NCORES = 8
TOK = N_TOKENS // NCORES  # tokens per core
P = 128                   # partitions
KT = D_IN // P            # contraction tiles
TT = TOK // P             # token tiles per core
FH = 512                  # max fp32 matmul free dim (one PSUM bank)

# "f32r" (default): fp32 data, PE in float32r mode (fast, ~fp32 precision)
# "f32": strict fp32 matmuls (4x slower PE)
# "bf16": bf16 inputs (half DMA traffic, ~3e-3 rel err)
MODE = os.environ.get("MOE_KERNEL_MODE", "f32r")

_F32 = mybir.dt.float32
_F32R = mybir.dt.float32r
_BF16 = mybir.dt.bfloat16


class _ChunkedDrainTileContext(tile.TileContext):
    """TileContext adapted to a walrus that allows ONE sync wait per
    instruction ("Too many sync wait commands", CoreV3GenImpl setupSyncWait).

    Stock Tile attaches up to ~3 waits to an instruction (and the whole
    global-clock wait set to the tail drain). Every extra wait is hoisted
    onto a same-engine InstNoOp carrier emitted immediately before the
    instruction, so the engine's sequencer observes the sems in order.
    """

    _HOIST_WAITS = os.environ.get("MOE_HOIST_WAITS", "0") == "1"

    def __init__(self, *a, **kw):
        super().__init__(*a, **kw)
        self._last_by_engine = {}

    # Walrus setupSyncWait allows exactly ONE wait per instruction on this
    # toolchain — including NoOp/Drain (probed: 2 and 4 both rejected).
    _NOOP_WAITS = int(os.environ.get("MOE_NOOP_WAITS", "1"))

    def _add_instruction(self, inst):
        si = getattr(inst, "sync_info", None)
        cap = (self._NOOP_WAITS
               if isinstance(inst, (mybir.InstNoOp, mybir.InstDrain)) else 1)
        if si is not None and si.on_wait and len(si.on_wait) > cap:
            waits = list(si.on_wait)
            if cap > 1:
                head, waits = waits[:-cap], waits[-cap:]
                for g0 in range(0, len(head), cap):
                    g = head[g0:g0 + cap]
                    nop = mybir.InstNoOp(
                        name=self.nc.get_next_instruction_name(), ins=[],
                        outs=[])
                    nop.engine = inst.engine
                    nop.bass_nofuse = True
                    nop.sync_info = bass_rust.SyncInfo(on_wait=g, on_update=[])
                    super()._add_instruction(nop)
                inst.sync_info = bass_rust.SyncInfo(
                    on_wait=waits, on_update=list(si.on_update or []))
                self._last_by_engine[inst.engine] = inst
                super()._add_instruction(inst)
                return
            # Optionally park one extra wait on the immediately preceding
            # same-engine instruction when it carries no waits/updates of its
            # own: the wait just fires one slot earlier in the same stream.
            if self._HOIST_WAITS and len(waits) == 2:
                prev = self._last_by_engine.get(inst.engine)
                psi = getattr(prev, "sync_info", None) if prev is not None else None
                if prev is not None and (
                    psi is None or (not psi.on_wait and not psi.on_update)
                ):
                    prev.sync_info = bass_rust.SyncInfo(
                        on_wait=[waits[0]], on_update=[])
                    waits = waits[1:]
            for w in waits[:-1]:
                nop = mybir.InstNoOp(
                    name=self.nc.get_next_instruction_name(), ins=[], outs=[]
                )
                nop.engine = inst.engine
                nop.bass_nofuse = True
                nop.sync_info = bass_rust.SyncInfo(on_wait=[w], on_update=[])
                super()._add_instruction(nop)
            inst.sync_info = bass_rust.SyncInfo(
                on_wait=[waits[-1]], on_update=list(si.on_update or [])
            )
        self._last_by_engine[inst.engine] = inst
        super()._add_instruction(inst)

    def _drain_and_barrier(self, tick_clock, wait_clock):
        drain_inst = self.nc.sync.drain()
        wait_clock.add_sem_waits(
            drain_inst.ins, ScopedClock({None: tick_clock.global_clock})
        )
        si = drain_inst.ins.sync_info
        waits = list(si.on_wait or []) if si is not None else []
        if len(waits) > 1:
            drain_inst.ins.sync_info = bass_rust.SyncInfo(
                on_wait=waits[:1], on_update=list(si.on_update or [])
            )
            for w in waits[1:]:
                extra = self.nc.sync.drain()
                extra.ins.sync_info = bass_rust.SyncInfo(on_wait=[w], on_update=[])

        self.nc.all_engine_barrier()
        assert self.sems is not None
        popped = self.nc._tile_sem_poison_stack.pop()
        assert popped is self._sem_poison
        self.nc.clear_and_free_semaphores(list(self.sems.allocated().values()))
        self.nc.all_engine_barrier()


def build_nc(mode: str = MODE, reps: int = 1, internal_io: bool = False,
             n_experts: int = E, do_gate: bool = True, do_bias: bool = True,
             do_fma: bool = True, do_store: bool = True, do_mm: bool = True,
             we_loads: int | None = None, fh: int = FH,
             hw_loop: bool = False) -> bass.Bass:
    """Build the per-core Bass program.

    reps: repeat the compute body (timing harnesses amortize dispatch
    overhead); internal_io: inputs live in internal DRAM seeded on-device
    (timing without host transfers); remaining flags ablate stages.
    """
    mmdt = {"bf16": _BF16, "f32": _F32, "f32r": _F32R}[mode]

    nc = bass.Bass()
    kind_in = {} if internal_io else {"kind": "ExternalInput"}
    xT_d = nc.dram_tensor("xT", [D_IN, TOK], mmdt, **kind_in)
    We_d = nc.dram_tensor("We", [E, D_IN, D_OUT], mmdt, **kind_in)
    be_d = nc.dram_tensor("be", [E, D_OUT], mmdt, **kind_in)
    Wg_d = nc.dram_tensor("Wg", [D_IN, E], mmdt, **kind_in)
    bg_d = nc.dram_tensor("bg", [E], _F32, **kind_in)
    if internal_io:
        out_d = nc.dram_tensor("out", [TOK, D_OUT], _F32)
        probe_d = nc.dram_tensor("probe", [P, P], _F32, kind="ExternalOutput")
    else:
        out_d = nc.dram_tensor("out", [TOK, D_OUT], _F32, kind="ExternalOutput")
        probe_d = None

    with _ChunkedDrainTileContext(nc) as tc, ExitStack() as ctx:
        singles = ctx.enter_context(tc.tile_pool(name="singles", bufs=1))
        wepool = ctx.enter_context(tc.tile_pool(name="we", bufs=2))
        # PSUM budget: pair pool 2x[128,1024] = 4 banks; ps_b holds the gate
        # logits tile (2 banks) + the exp-transpose staging tile (1 bank).
        ps_pair = ctx.enter_context(tc.tile_pool(name="ps_pair", bufs=2,
                                                 space="PSUM"))
        ps_b = ctx.enter_context(tc.tile_pool(name="ps_b", bufs=1,
                                              space="PSUM"))

        if internal_io:
            # Seed internal inputs on-device with benign constants (values
            # are irrelevant to timing; softmax logits stay small/finite).
            seed = singles.tile([P, D_OUT], _F32, tag="seed")
            nc.vector.memset(seed[:], 0.005)
            if mmdt != _F32:
                seedm = singles.tile([P, D_OUT], mmdt, tag="seedm")
                nc.scalar.copy(seedm[:], seed[:])
            else:
                seedm = seed

            def rep_src(n_rep):
                s = seedm[:, :].opt()
                return bass.AP(tensor=s.tensor, offset=s.offset,
                               ap=[[s.ap[0][0], P], [0, n_rep], [1, D_OUT]])

            nc.sync.dma_start(xT_d.rearrange("(k p) n -> p k n", p=P),
                              rep_src(KT))
            for e in range(E):
                nc.sync.dma_start(We_d[e].rearrange("(k p) o -> p k o", p=P),
                                  rep_src(KT))
            nc.sync.dma_start(be_d[:, :], seedm[0:E, :])
            nc.sync.dma_start(Wg_d.rearrange("(k p) e -> p k e", p=P),
                              seedm[:, 0:KT * E].rearrange(
                                  "p (k e) -> p k e", k=KT))
            nc.sync.dma_start(bg_d[:], seed[0, 0:E])

        # small identity for the [E,tok] -> [tok,E] exp transpose
        ident8f = singles.tile([E, E], _F32, tag="id8f")
        make_identity(nc, ident8f)
        if mmdt != _F32:
            ident8 = singles.tile([E, E], mmdt, tag="id8")
            nc.scalar.copy(ident8[:], ident8f[:])
        else:
            ident8 = ident8f

        wg_sb = singles.tile([P, KT, E], mmdt, tag="wg")
        nc.sync.dma_start(wg_sb[:], Wg_d.rearrange("(k p) e -> p k e", p=P))
        bg_col = singles.tile([E, 1], _F32, tag="bg")
        nc.sync.dma_start(bg_col[:], bg_d[:])
        be_sb = singles.tile([E, D_OUT], mmdt, tag="be")
        nc.sync.dma_start(be_sb[:], be_d[:, :])

        def _rep_body():
            xT = singles.tile([P, KT, TOK], mmdt, tag="xT")
            nc.sync.dma_start(xT[:], xT_d.rearrange("(k p) n -> p k n", p=P))
            acc = singles.tile([P, TT, D_OUT], _F32, tag="acc")

            if do_gate:
                # logits^T [E, tok] in PSUM (E on partitions), bias add as a
                # per-partition scalar, exp over all tokens at once.
                pg = ps_b.tile([E, TOK], _F32, tag="g")
                for k in range(KT):
                    for h in range(TOK // FH):
                        nc.tensor.matmul(
                            pg[:, h * FH:(h + 1) * FH], wg_sb[:, k, :],
                            xT[:, k, h * FH:(h + 1) * FH],
                            start=(k == 0), stop=(k == KT - 1),
                        )
                ltT = singles.tile([E, TOK], _F32, tag="ltT")
                nc.vector.tensor_scalar_add(ltT[:], pg[:], bg_col[:])
                expT = singles.tile([E, TOK], mmdt, tag="expT")
                nc.scalar.activation(expT[:], ltT[:],
                                     mybir.ActivationFunctionType.Exp)
                # transpose exp to token layout: 8 blocks into one PSUM bank
                ptr = ps_b.tile([P, TT * E], mmdt, tag="tr")
                for i in range(TT):
                    nc.tensor.transpose(ptr[:, i * E:(i + 1) * E],
                                        expT[:, i * P:(i + 1) * P], ident8[:])
                exp_tok = singles.tile([P, TT, E], _F32, tag="exptok")
                nc.scalar.copy(exp_tok.rearrange("p a b -> p (a b)"), ptr[:])
                s_tok = singles.tile([P, TT, 1], _F32, tag="stok")
                nc.vector.reduce_sum(s_tok[:], exp_tok[:],
                                     axis=mybir.AxisListType.X)
                r_tok = singles.tile([P, TT, 1], _F32, tag="rtok")
                nc.vector.reciprocal(r_tok[:], s_tok[:])

            if do_bias:
                # acc init: (unnormalized) exp @ be
                for i in range(TT):
                    pb = ps_pair.tile([P, D_OUT], _F32, tag="pair")
                    for h in range(D_OUT // FH):
                        nc.tensor.matmul(
                            pb[:, h * FH:(h + 1) * FH],
                            expT[:, i * P:(i + 1) * P],
                            be_sb[:, h * FH:(h + 1) * FH],
                            start=True, stop=True,
                        )
                    nc.scalar.copy(acc[:, i, :], pb[:])

            # experts: acc += exp[:, e] * (x @ We[e])
            we = None
            for e in range(n_experts):
                if we_loads is None or e < we_loads:
                    we = wepool.tile([P, KT, D_OUT], mmdt, tag="we")
                    nc.sync.dma_start(
                        we[:], We_d[e].rearrange("(k p) o -> p k o", p=P))
                for i in range(TT if do_mm else 0):
                    isl = slice(i * P, (i + 1) * P)
                    pm = ps_pair.tile([P, D_OUT], _F32, tag="pair")
                    for k in range(KT):
                        for h in range(D_OUT // fh):
                            nc.tensor.matmul(
                                pm[:, h * fh:(h + 1) * fh], xT[:, k, isl],
                                we[:, k, h * fh:(h + 1) * fh],
                                start=(k == 0), stop=(k == KT - 1),
                            )
                    if do_fma:
                        nc.vector.scalar_tensor_tensor(
                            out=acc[:, i, :], in0=pm[:],
                            scalar=exp_tok[:, i, e:e + 1], in1=acc[:, i, :],
                            op0=mybir.AluOpType.mult, op1=mybir.AluOpType.add,
                        )

            if do_gate and do_fma:
                # normalize: acc *= 1/sum(exp), broadcast over D_OUT
                r = r_tok[:, :, 0:1].opt()
                rb = bass.AP(tensor=r.tensor, offset=r.offset,
                             ap=[r.ap[0], r.ap[1], [0, D_OUT]])
                nc.vector.tensor_mul(acc[:], acc[:], rb)

            if do_store:
                nc.sync.dma_start(out_d.rearrange("(i p) o -> p i o", p=P),
                                  acc[:])
            return acc

        if hw_loop:
            with tc.For_i(0, reps, 1):
                acc = _rep_body()
        else:
            for _ in range(reps):
                acc = _rep_body()

        if internal_io:
            nc.sync.dma_start(probe_d[:, :], acc[:, 0, 0:P])

    return nc


def build_nc_v2(mode: str = "bf16", reps: int = 1, internal_io: bool = False,
                stock_tc: bool = False) -> bass.Bass:
    """Instruction-lean build: the 64 (expert, token-tile) GEMM iterations
    run in two nested tc.For_i hardware loops, so the static program is a
    few hundred instructions instead of a few thousand.

    Constraints discovered on this toolchain that shape the design:
      - ldweights (matmul stationary operand) cannot take register offsets,
        so the stationary x token-tile is staged into a fixed SBUF tile by
        DMA (which can).
      - register-offset DMA must source from DRAM (SBUF needs bacc), so
        x is staged from a token-tile-major DRAM copy (xT2) and exp^T from
        a DRAM scratch dump.
      - matmul PSUM output is capped at one bank (512 f32), so every
        matmul writes F=512 halves.
      - register allocation has no spilling and registers leak across
        loop constructs, so dynamic APs are kept to a handful per rep
        (all matmul operands are static APs).
      - walrus allows ONE sync wait per compute instruction
        (_ChunkedDrainTileContext hoists extras onto NoOp carriers).

    Per-expert weights are staged from HBM once per expert (16 MB/rep
    total, same traffic as a full preload); x tiles are restaged per
    (expert, tile) at 32 KiB each (2 MB/rep, contiguous 2 KiB lines).
    """
    assert mode == "bf16"
    mmdt = _BF16

    nc = bass.Bass()
    kind_in = {} if internal_io else {"kind": "ExternalInput"}
    # x shard, token-tile-major: xT2[i, k, n] = x[i*128 + n, k]
    xT2_d = nc.dram_tensor("xT2", [TT, D_IN, P], mmdt, **kind_in)
    We_d = nc.dram_tensor("We", [E, D_IN, D_OUT], mmdt, **kind_in)
    be_d = nc.dram_tensor("be", [E, D_OUT], mmdt, **kind_in)
    Wg_d = nc.dram_tensor("Wg", [D_IN, E], mmdt, **kind_in)
    bg_d = nc.dram_tensor("bg", [E], _F32, **kind_in)
    if internal_io:
        out_d = nc.dram_tensor("out", [TOK, D_OUT], _F32)
        probe_d = nc.dram_tensor("probe", [P, P], _F32, kind="ExternalOutput")
    else:
        out_d = nc.dram_tensor("out", [TOK, D_OUT], _F32, kind="ExternalOutput")
        probe_d = None
    # DRAM views. x/We use a permuted contraction layout: partition p holds
    # rows 8p..8p+7 (j=0..7); both operands use the same permutation so the
    # contraction is unchanged. Every partition line is contiguous DRAM.
    xT2_v = xT2_d.rearrange("t (p j) n -> p t j n", p=P)       # [128,8,8,128]
    We_v = We_d.rearrange("e (p j) o -> p e j o", p=P)         # [128,8,8,1024]
    out_v = out_d.rearrange("(i p) o -> p i o", p=P)

    # Fewer DMA-completion semaphore lanes shrink the global wait set that
    # every loop exit/reset barrier (and its single-wait NoOp chunks) must
    # observe: 4 engine sems + N DMAHW lanes. Scoped to this build.
    import concourse.tile_sem_assignment as _tsa
    n_lanes = int(os.environ.get("MOE_DMA_LANES", "2"))
    saved_lanes = _tsa.NUM_HWDGE_SEMS
    _tsa.NUM_HWDGE_SEMS = n_lanes

    tc_cls = tile.TileContext if stock_tc else _ChunkedDrainTileContext
    try:
        return _build_nc_v2_body(nc, tc_cls, internal_io, reps, mmdt,
                                 xT2_d, We_d, be_d, Wg_d, bg_d, out_d,
                                 probe_d, xT2_v, We_v, out_v)
    finally:
        _tsa.NUM_HWDGE_SEMS = saved_lanes


def _build_nc_v2_body(nc, tc_cls, internal_io, reps, mmdt, xT2_d, We_d,
                      be_d, Wg_d, bg_d, out_d, probe_d, xT2_v, We_v, out_v):
    with tc_cls(nc) as tc, ExitStack() as ctx:
        singles = ctx.enter_context(tc.tile_pool(name="singles", bufs=1))
        # PSUM: pair pool 2x[128,1024] = 4 banks; ps_b holds gate logits
        # (2 banks) + exp-transpose staging (1 bank).
        ps_pair = ctx.enter_context(tc.tile_pool(name="ps_pair", bufs=2,
                                                 space="PSUM"))
        ps_b = ctx.enter_context(tc.tile_pool(name="ps_b", bufs=1,
                                              space="PSUM"))

        if internal_io:
            seed = singles.tile([P, D_OUT], _F32, tag="seed")
            nc.vector.memset(seed[:], 0.005)
            seedm = singles.tile([P, D_OUT], mmdt, tag="seedm")
            nc.scalar.copy(seedm[:], seed[:])

            def rep_src(n_rep, n_inner):
                s = seedm[:, :].opt()
                return bass.AP(tensor=s.tensor, offset=s.offset,
                               ap=[[s.ap[0][0], P], [0, n_rep], [1, n_inner]])

            for t in range(TT):
                nc.sync.dma_start(
                    xT2_d[t].rearrange("(p j) n -> p j n", p=P),
                    rep_src(KT, P))
            for e in range(E):
                nc.sync.dma_start(We_v[:, e], rep_src(KT, D_OUT))
            nc.sync.dma_start(be_d[:, :], seedm[0:E, :])
            nc.sync.dma_start(Wg_d.rearrange("(k p) e -> p k e", p=P),
                              seedm[:, 0:KT * E].rearrange(
                                  "p (k e) -> p k e", k=KT))
            nc.sync.dma_start(bg_d[:], seed[0, 0:E])

        # --- setup (outside the rep loop; compiled once, not per rep) ---
        ident8f = singles.tile([E, E], _F32, tag="id8f")
        make_identity(nc, ident8f)
        ident8 = singles.tile([E, E], mmdt, tag="id8")
        nc.scalar.copy(ident8[:], ident8f[:])

        # Wg in the same permuted-k layout (partition p = rows 8p..8p+7)
        wg_sb = singles.tile([P, KT, E], mmdt, tag="wg")
        nc.sync.dma_start(wg_sb[:], Wg_d.rearrange("(p j) e -> p j e", p=P))
        bg_col = singles.tile([E, 1], _F32, tag="bg")
        nc.sync.dma_start(bg_col[:], bg_d[:])
        be_sb = singles.tile([E, D_OUT], mmdt, tag="be")
        nc.sync.dma_start(be_sb[:], be_d[:, :])

        def _rep_body():
            # x for the gate in one contiguous load: [p, i, j, n]
            xg = singles.tile([P, TT, KT, P], mmdt, tag="xg")
            nc.sync.dma_start(xg[:], xT2_v)

            # --- gate: logits^T [E, tok] with E on partitions, tok=(i,n) ---
            pg = ps_b.tile([E, TT, P], _F32, tag="g")
            for j in range(KT):
                for h in range(2):
                    nc.tensor.matmul(
                        pg[:, h * (TT // 2):(h + 1) * (TT // 2), :],
                        wg_sb[:, j, :],
                        xg[:, h * (TT // 2):(h + 1) * (TT // 2), j, :],
                        start=(j == 0), stop=(j == KT - 1))
            ltT = singles.tile([E, TOK], _F32, tag="ltT")
            nc.vector.tensor_scalar_add(ltT[:], pg.rearrange("e a b -> e (a b)"),
                                        bg_col[:])
            expT = singles.tile([E, TOK], mmdt, tag="expT")
            nc.scalar.activation(expT[:], ltT[:],
                                 mybir.ActivationFunctionType.Exp)
            # transpose exp to token layout: [P, TT*E], col i*E+e
            ptr = ps_b.tile([P, TT * E], mmdt, tag="tr")
            for i in range(TT):
                nc.tensor.transpose(ptr[:, i * E:(i + 1) * E],
                                    expT[:, i * P:(i + 1) * P], ident8[:])
            exp_tok = singles.tile([P, TT, E], _F32, tag="exptok")
            nc.vector.tensor_copy(exp_tok.rearrange("p a b -> p (a b)"),
                                  ptr[:])
            s_tok = singles.tile([P, TT, 1], _F32, tag="stok")
            nc.vector.reduce_sum(s_tok[:], exp_tok[:],
                                 axis=mybir.AxisListType.X)
            r_tok = singles.tile([P, TT, 1], _F32, tag="rtok")
            nc.vector.reciprocal(r_tok[:], s_tok[:])

            # --- bias init (static): acc[i] = exp-tile^T @ be per tile ---
            acc = singles.tile([P, TT, D_OUT], _F32, tag="acc")
            for i in range(TT):
                pb = ps_pair.tile([P, D_OUT], _F32, tag="pair")
                for h in range(2):
                    nc.tensor.matmul(pb[:, h * FH:(h + 1) * FH],
                                     expT[:, i * P:(i + 1) * P],
                                     be_sb[:, h * FH:(h + 1) * FH],
                                     start=True, stop=True)
                nc.vector.tensor_copy(acc[:, i, :], pb[:])

            exp_flat = exp_tok.rearrange("p a b -> p (a b)")
            acc_flat = acc.rearrange("p a b -> p (a b)")

            # --- main loops: experts outer (weights staged once per e),
            # token tiles inner (x tile staged per iteration). All matmul
            # operands are static APs; only DMA/DVE use register offsets.
            with tc.For_i(0, E, 1) as e:
                westage = singles.tile([P, KT, D_OUT], mmdt, tag="westage")
                nc.sync.dma_start(westage[:], We_v[:, bass.ds(e, 1), :, :])
                with tc.For_i(0, TT, 1) as i:
                    xstage = singles.tile([P, KT, P], mmdt, tag="xstage")
                    nc.sync.dma_start(xstage[:],
                                      xT2_v[:, bass.ds(i, 1), :, :])
                    pm = ps_pair.tile([P, D_OUT], _F32, tag="pair")
                    for j in range(KT):
                        for h in range(2):
                            nc.tensor.matmul(
                                pm[:, h * FH:(h + 1) * FH], xstage[:, j, :],
                                westage[:, j, h * FH:(h + 1) * FH],
                                start=(j == 0), stop=(j == KT - 1),
                            )
                    nc.vector.scalar_tensor_tensor(
                        out=acc_flat[:, bass.ds(i * D_OUT, D_OUT)], in0=pm[:],
                        scalar=exp_flat[:, bass.ds(i * E + e, 1)],
                        in1=acc_flat[:, bass.ds(i * D_OUT, D_OUT)],
                        op0=mybir.AluOpType.mult, op1=mybir.AluOpType.add,
                    )

            # normalize: acc *= 1/sum(exp), broadcast over D_OUT
            r = r_tok[:, :, 0:1].opt()
            rb = bass.AP(tensor=r.tensor, offset=r.offset,
                         ap=[r.ap[0], r.ap[1], [0, D_OUT]])
            nc.vector.tensor_mul(acc[:], acc[:], rb)
            nc.sync.dma_start(out_v, acc[:])
            return acc

        for _ in range(reps):
            acc_last = _rep_body()

        if internal_io:
            nc.sync.dma_start(probe_d[:, :], acc_last[:, 0, 0:P])

    return nc


_NC_CACHE: dict = {}


def _get_nc(mode: str, reps: int = 1) -> bass.Bass:
    key = (mode, reps)
    if key not in _NC_CACHE:
        _NC_CACHE[key] = build_nc(mode, reps)
    return _NC_CACHE[key]


def make_in_maps(x, We, be, Wg, bg, mode: str = MODE):
    import ml_dtypes

    dt_np = ml_dtypes.bfloat16 if mode == "bf16" else np.float32
    We_c = np.ascontiguousarray(We, dtype=dt_np)
    be_c = np.ascontiguousarray(be, dtype=dt_np)
    Wg_c = np.ascontiguousarray(Wg, dtype=dt_np)
    bg_c = np.ascontiguousarray(bg, dtype=np.float32)
    in_maps = []
    for c in range(NCORES):
        xs = np.asarray(x[c * TOK:(c + 1) * TOK], dtype=dt_np)
        # token-tile-major transpose: xT2[i, k, n] = x[i*128 + n, k]
        xT2 = np.ascontiguousarray(
            xs.reshape(TT, P, D_IN).transpose(0, 2, 1))
        in_maps.append({
            "xT2": xT2,
            "We": We_c,
            "be": be_c,
            "Wg": Wg_c,
            "bg": bg_c,
        })
    return in_maps


def kernel(x, We, be, Wg, bg):
    key = ("v2", MODE)
    if key not in _NC_CACHE:
        _NC_CACHE[key] = build_nc_v2("bf16", reps=1)
    nc = _NC_CACHE[key]
    in_maps = make_in_maps(x, We, be, Wg, bg, "bf16")
    res = run_bass_kernel_spmd(nc, in_maps, list(range(NCORES)))
    out = np.concatenate([res.results[c]["out"] for c in range(NCORES)], axis=0)
    return out.astype(np.float32)



# revision 39
# speedup vs baseline: 3.6905x; 3.6905x over previous
"""Trainium2 Bass kernel for the dense MoE layer (nn_MoELayer_74371653698164).

Reference computation (fp32):
    gate  = softmax(x @ Wg + bg)                    # [N, E]
    out   = sum_e gate[:, e] * (x @ We[e] + be[e])  # [N, D_OUT]

Shipped kernel: build_nc_v2 (bf16, hardware-looped; see its docstring).
Data-parallel over tokens: each of the 8 cores gets N/8 = 1024 tokens and
the full expert/gate weights, no collectives. The dominant cost in this
harness scales with the STATIC instruction count of the program (per-call
NEFF compile/load, ~40 us/instruction), so the 64 (expert, token-tile)
GEMM iterations run in two nested tc.For_i hardware loops instead of
being unrolled: ~420 static instructions per rep instead of ~2300.

build_nc below is the earlier fully-unrolled f32r variant, kept for A/B.

Original baseline notes:
  - Data-parallel over tokens: each of the 8 cores gets N/8 = 1024 tokens and
    the full expert/gate weights. No collectives.
  - x is pre-transposed on the host (a weights-style layout change), so the
    contraction dim lands on partitions with a single DMA and no on-device
    transpose pass.
  - Softmax is factored: out = r * (sum_e exp_e * (x @ We[e] + be[e])) with
    r = 1 / sum_e exp_e. Logits are computed TRANSPOSED ([E, tok], E on
    partitions) so the bias add is a per-partition scalar op and exp is one
    activation over all tokens; logits here are ~N(0,1) so max-subtraction
    is unnecessary in fp32.
  - Per expert: stream We[e] from HBM, run 128x512 matmuls into a 2-bank
    [128,1024] PSUM pair, and fold the gate in with one DVE FMA per
    (expert, token-tile): acc = psum * exp[:, e] + acc. The bias term
    (exp @ be, one K=8 matmul pair per token tile) initializes acc.
  - Matmul operands are float32r: full fp32 data that the PE streams at
    1 cycle/row for N>=256 (vs 4 for strict fp32), near-fp32 precision.
  - Instruction count is minimized throughout (batched drains, paired PSUM
    banks, single big DMAs) — both dispatch overhead and real-HW sync cost
    scale with it.

kernel(**inputs) takes the FULL unsharded inputs and returns the FULL output.
"""
import os
from contextlib import ExitStack

import numpy as np

import bass_rust
import concourse.bass as bass
import concourse.mybir as mybir
import concourse.tile as tile
from concourse.bass_utils import run_bass_kernel_spmd
from concourse.masks import make_identity
from concourse.vector_clock import ScopedClock

# Problem shape (hardcoded per harness contract).
N_TOKENS, D_IN, D_OUT, E = 8192, 1024, 1024, 8
NCORES = 8
TOK = N_TOKENS // NCORES  # tokens per core
P = 128                   # partitions
KT = D_IN // P            # contraction tiles
TT = TOK // P             # token tiles per core
FH = 512                  # max fp32 matmul free dim (one PSUM bank)

# "f32r" (default): fp32 data, PE in float32r mode (fast, ~fp32 precision)
# "f32": strict fp32 matmuls (4x slower PE)
# "bf16": bf16 inputs (half DMA traffic, ~3e-3 rel err)
MODE = os.environ.get("MOE_KERNEL_MODE", "f32r")

_F32 = mybir.dt.float32
_F32R = mybir.dt.float32r
_BF16 = mybir.dt.bfloat16


class _ChunkedDrainTileContext(tile.TileContext):
    """TileContext adapted to a walrus that allows ONE sync wait per
    instruction ("Too many sync wait commands", CoreV3GenImpl setupSyncWait).

    Stock Tile attaches up to ~3 waits to an instruction (and the whole
    global-clock wait set to the tail drain). Every extra wait is hoisted
    onto a same-engine InstNoOp carrier emitted immediately before the
    instruction, so the engine's sequencer observes the sems in order.
    """

    _HOIST_WAITS = os.environ.get("MOE_HOIST_WAITS", "0") == "1"

    def __init__(self, *a, **kw):
        super().__init__(*a, **kw)
        self._last_by_engine = {}

    # Walrus setupSyncWait allows exactly ONE wait per instruction on this
    # toolchain — including NoOp/Drain (probed: 2 and 4 both rejected).
    _NOOP_WAITS = int(os.environ.get("MOE_NOOP_WAITS", "1"))

    def _add_instruction(self, inst):
        si = getattr(inst, "sync_info", None)
        cap = (self._NOOP_WAITS
               if isinstance(inst, (mybir.InstNoOp, mybir.InstDrain)) else 1)
        if si is not None and si.on_wait and len(si.on_wait) > cap:
            waits = list(si.on_wait)
            if cap > 1:
                head, waits = waits[:-cap], waits[-cap:]
                for g0 in range(0, len(head), cap):
                    g = head[g0:g0 + cap]
                    nop = mybir.InstNoOp(
                        name=self.nc.get_next_instruction_name(), ins=[],
                        outs=[])
                    nop.engine = inst.engine
                    nop.bass_nofuse = True
                    nop.sync_info = bass_rust.SyncInfo(on_wait=g, on_update=[])
                    super()._add_instruction(nop)
                inst.sync_info = bass_rust.SyncInfo(
                    on_wait=waits, on_update=list(si.on_update or []))
                self._last_by_engine[inst.engine] = inst
                super()._add_instruction(inst)
                return
            # Optionally park one extra wait on the immediately preceding
            # same-engine instruction when it carries no waits/updates of its
            # own: the wait just fires one slot earlier in the same stream.
            if self._HOIST_WAITS and len(waits) == 2:
                prev = self._last_by_engine.get(inst.engine)
                psi = getattr(prev, "sync_info", None) if prev is not None else None
                if prev is not None and (
                    psi is None or (not psi.on_wait and not psi.on_update)
                ):
                    prev.sync_info = bass_rust.SyncInfo(
                        on_wait=[waits[0]], on_update=[])
                    waits = waits[1:]
            for w in waits[:-1]:
                nop = mybir.InstNoOp(
                    name=self.nc.get_next_instruction_name(), ins=[], outs=[]
                )
                nop.engine = inst.engine
                nop.bass_nofuse = True
                nop.sync_info = bass_rust.SyncInfo(on_wait=[w], on_update=[])
                super()._add_instruction(nop)
            inst.sync_info = bass_rust.SyncInfo(
                on_wait=[waits[-1]], on_update=list(si.on_update or [])
            )
        self._last_by_engine[inst.engine] = inst
        super()._add_instruction(inst)

    def _drain_and_barrier(self, tick_clock, wait_clock):
        drain_inst = self.nc.sync.drain()
        wait_clock.add_sem_waits(
            drain_inst.ins, ScopedClock({None: tick_clock.global_clock})
        )
        si = drain_inst.ins.sync_info
        waits = list(si.on_wait or []) if si is not None else []
        if len(waits) > 1:
            drain_inst.ins.sync_info = bass_rust.SyncInfo(
                on_wait=waits[:1], on_update=list(si.on_update or [])
            )
            for w in waits[1:]:
                extra = self.nc.sync.drain()
                extra.ins.sync_info = bass_rust.SyncInfo(on_wait=[w], on_update=[])

        self.nc.all_engine_barrier()
        assert self.sems is not None
        popped = self.nc._tile_sem_poison_stack.pop()
        assert popped is self._sem_poison
        self.nc.clear_and_free_semaphores(list(self.sems.allocated().values()))
        self.nc.all_engine_barrier()


def build_nc(mode: str = MODE, reps: int = 1, internal_io: bool = False,
             n_experts: int = E, do_gate: bool = True, do_bias: bool = True,
             do_fma: bool = True, do_store: bool = True, do_mm: bool = True,
             we_loads: int | None = None, fh: int = FH,
             hw_loop: bool = False) -> bass.Bass:
    """Build the per-core Bass program.

    reps: repeat the compute body (timing harnesses amortize dispatch
    overhead); internal_io: inputs live in internal DRAM seeded on-device
    (timing without host transfers); remaining flags ablate stages.
    """
    mmdt = {"bf16": _BF16, "f32": _F32, "f32r": _F32R}[mode]

    nc = bass.Bass()
    kind_in = {} if internal_io else {"kind": "ExternalInput"}
    xT_d = nc.dram_tensor("xT", [D_IN, TOK], mmdt, **kind_in)
    We_d = nc.dram_tensor("We", [E, D_IN, D_OUT], mmdt, **kind_in)
    be_d = nc.dram_tensor("be", [E, D_OUT], mmdt, **kind_in)
    Wg_d = nc.dram_tensor("Wg", [D_IN, E], mmdt, **kind_in)
    bg_d = nc.dram_tensor("bg", [E], _F32, **kind_in)
    if internal_io:
        out_d = nc.dram_tensor("out", [TOK, D_OUT], _F32)
        probe_d = nc.dram_tensor("probe", [P, P], _F32, kind="ExternalOutput")
    else:
        out_d = nc.dram_tensor("out", [TOK, D_OUT], _F32, kind="ExternalOutput")
        probe_d = None

    with _ChunkedDrainTileContext(nc) as tc, ExitStack() as ctx:
        singles = ctx.enter_context(tc.tile_pool(name="singles", bufs=1))
        wepool = ctx.enter_context(tc.tile_pool(name="we", bufs=2))
        # PSUM budget: pair pool 2x[128,1024] = 4 banks; ps_b holds the gate
        # logits tile (2 banks) + the exp-transpose staging tile (1 bank).
        ps_pair = ctx.enter_context(tc.tile_pool(name="ps_pair", bufs=2,
                                                 space="PSUM"))
        ps_b = ctx.enter_context(tc.tile_pool(name="ps_b", bufs=1,
                                              space="PSUM"))

        if internal_io:
            # Seed internal inputs on-device with benign constants (values
            # are irrelevant to timing; softmax logits stay small/finite).
            seed = singles.tile([P, D_OUT], _F32, tag="seed")
            nc.vector.memset(seed[:], 0.005)
            if mmdt != _F32:
                seedm = singles.tile([P, D_OUT], mmdt, tag="seedm")
                nc.scalar.copy(seedm[:], seed[:])
            else:
                seedm = seed

            def rep_src(n_rep):
                s = seedm[:, :].opt()
                return bass.AP(tensor=s.tensor, offset=s.offset,
                               ap=[[s.ap[0][0], P], [0, n_rep], [1, D_OUT]])

            nc.sync.dma_start(xT_d.rearrange("(k p) n -> p k n", p=P),
                              rep_src(KT))
            for e in range(E):
                nc.sync.dma_start(We_d[e].rearrange("(k p) o -> p k o", p=P),
                                  rep_src(KT))
            nc.sync.dma_start(be_d[:, :], seedm[0:E, :])
            nc.sync.dma_start(Wg_d.rearrange("(k p) e -> p k e", p=P),
                              seedm[:, 0:KT * E].rearrange(
                                  "p (k e) -> p k e", k=KT))
            nc.sync.dma_start(bg_d[:], seed[0, 0:E])

        # small identity for the [E,tok] -> [tok,E] exp transpose
        ident8f = singles.tile([E, E], _F32, tag="id8f")
        make_identity(nc, ident8f)
        if mmdt != _F32:
            ident8 = singles.tile([E, E], mmdt, tag="id8")
            nc.scalar.copy(ident8[:], ident8f[:])
        else:
            ident8 = ident8f

        wg_sb = singles.tile([P, KT, E], mmdt, tag="wg")
        nc.sync.dma_start(wg_sb[:], Wg_d.rearrange("(k p) e -> p k e", p=P))
        bg_col = singles.tile([E, 1], _F32, tag="bg")
        nc.sync.dma_start(bg_col[:], bg_d[:])
        be_sb = singles.tile([E, D_OUT], mmdt, tag="be")
        nc.sync.dma_start(be_sb[:], be_d[:, :])

        def _rep_body():
            xT = singles.tile([P, KT, TOK], mmdt, tag="xT")
            nc.sync.dma_start(xT[:], xT_d.rearrange("(k p) n -> p k n", p=P))
            acc = singles.tile([P, TT, D_OUT], _F32, tag="acc")

            if do_gate:
                # logits^T [E, tok] in PSUM (E on partitions), bias add as a
                # per-partition scalar, exp over all tokens at once.
                pg = ps_b.tile([E, TOK], _F32, tag="g")
                for k in range(KT):
                    for h in range(TOK // FH):
                        nc.tensor.matmul(
                            pg[:, h * FH:(h + 1) * FH], wg_sb[:, k, :],
                            xT[:, k, h * FH:(h + 1) * FH],
                            start=(k == 0), stop=(k == KT - 1),
                        )
                ltT = singles.tile([E, TOK], _F32, tag="ltT")
                nc.vector.tensor_scalar_add(ltT[:], pg[:], bg_col[:])
                expT = singles.tile([E, TOK], mmdt, tag="expT")
                nc.scalar.activation(expT[:], ltT[:],
                                     mybir.ActivationFunctionType.Exp)
                # transpose exp to token layout: 8 blocks into one PSUM bank
                ptr = ps_b.tile([P, TT * E], mmdt, tag="tr")
                for i in range(TT):
                    nc.tensor.transpose(ptr[:, i * E:(i + 1) * E],
                                        expT[:, i * P:(i + 1) * P], ident8[:])
                exp_tok = singles.tile([P, TT, E], _F32, tag="exptok")
                nc.scalar.copy(exp_tok.rearrange("p a b -> p (a b)"), ptr[:])
                s_tok = singles.tile([P, TT, 1], _F32, tag="stok")
                nc.vector.reduce_sum(s_tok[:], exp_tok[:],
                                     axis=mybir.AxisListType.X)
                r_tok = singles.tile([P, TT, 1], _F32, tag="rtok")
                nc.vector.reciprocal(r_tok[:], s_tok[:])

            if do_bias:
                # acc init: (unnormalized) exp @ be
                for i in range(TT):
                    pb = ps_pair.tile([P, D_OUT], _F32, tag="pair")
                    for h in range(D_OUT // FH):
                        nc.tensor.matmul(
                            pb[:, h * FH:(h + 1) * FH],
                            expT[:, i * P:(i + 1) * P],
                            be_sb[:, h * FH:(h + 1) * FH],
                            start=True, stop=True,
                        )
                    nc.scalar.copy(acc[:, i, :], pb[:])

            # experts: acc += exp[:, e] * (x @ We[e])
            we = None
            for e in range(n_experts):
                if we_loads is None or e < we_loads:
                    we = wepool.tile([P, KT, D_OUT], mmdt, tag="we")
                    nc.sync.dma_start(
                        we[:], We_d[e].rearrange("(k p) o -> p k o", p=P))
                for i in range(TT if do_mm else 0):
                    isl = slice(i * P, (i + 1) * P)
                    pm = ps_pair.tile([P, D_OUT], _F32, tag="pair")
                    for k in range(KT):
                        for h in range(D_OUT // fh):
                            nc.tensor.matmul(
                                pm[:, h * fh:(h + 1) * fh], xT[:, k, isl],
                                we[:, k, h * fh:(h + 1) * fh],
                                start=(k == 0), stop=(k == KT - 1),
                            )
                    if do_fma:
                        nc.vector.scalar_tensor_tensor(
                            out=acc[:, i, :], in0=pm[:],
                            scalar=exp_tok[:, i, e:e + 1], in1=acc[:, i, :],
                            op0=mybir.AluOpType.mult, op1=mybir.AluOpType.add,
                        )

            if do_gate and do_fma:
                # normalize: acc *= 1/sum(exp), broadcast over D_OUT
                r = r_tok[:, :, 0:1].opt()
                rb = bass.AP(tensor=r.tensor, offset=r.offset,
                             ap=[r.ap[0], r.ap[1], [0, D_OUT]])
                nc.vector.tensor_mul(acc[:], acc[:], rb)

            if do_store:
                nc.sync.dma_start(out_d.rearrange("(i p) o -> p i o", p=P),
                                  acc[:])
            return acc

        if hw_loop:
            with tc.For_i(0, reps, 1):
                acc = _rep_body()
        else:
            for _ in range(reps):
                acc = _rep_body()

        if internal_io:
            nc.sync.dma_start(probe_d[:, :], acc[:, 0, 0:P])

    return nc


def build_nc_v2(mode: str = "bf16", reps: int = 1, internal_io: bool = False,
                stock_tc: bool = False) -> bass.Bass:
    """Instruction-lean build: the 64 (expert, token-tile) GEMM iterations
    run in two nested tc.For_i hardware loops, so the static program is a
    few hundred instructions instead of a few thousand.

    Constraints discovered on this toolchain that shape the design:
      - ldweights (matmul stationary operand) cannot take register offsets,
        so the stationary x token-tile is staged into a fixed SBUF tile by
        DMA (which can).
      - register-offset DMA must source from DRAM (SBUF needs bacc), so
        x is staged from a token-tile-major DRAM copy (xT2) and exp^T from
        a DRAM scratch dump.
      - matmul PSUM output is capped at one bank (512 f32), so every
        matmul writes F=512 halves.
      - register allocation has no spilling and registers leak across
        loop constructs, so dynamic APs are kept to a handful per rep
        (all matmul operands are static APs).
      - walrus allows ONE sync wait per compute instruction
        (_ChunkedDrainTileContext hoists extras onto NoOp carriers).

    Per-expert weights are staged from HBM once per expert (16 MB/rep
    total, same traffic as a full preload); x tiles are restaged per
    (expert, tile) at 32 KiB each (2 MB/rep, contiguous 2 KiB lines).
    """
    assert mode == "bf16"
    mmdt = _BF16

    nc = bass.Bass()
    kind_in = {} if internal_io else {"kind": "ExternalInput"}
    # x shard, token-tile-major: xT2[i, k, n] = x[i*128 + n, k]
    xT2_d = nc.dram_tensor("xT2", [TT, D_IN, P], mmdt, **kind_in)
    We_d = nc.dram_tensor("We", [E, D_IN, D_OUT], mmdt, **kind_in)
    be_d = nc.dram_tensor("be", [E, D_OUT], mmdt, **kind_in)
    Wg_d = nc.dram_tensor("Wg", [D_IN, E], mmdt, **kind_in)
    bg_d = nc.dram_tensor("bg", [E], _F32, **kind_in)
    if internal_io:
        out_d = nc.dram_tensor("out", [TOK, D_OUT], _F32)
        probe_d = nc.dram_tensor("probe", [P, P], _F32, kind="ExternalOutput")
    else:
        out_d = nc.dram_tensor("out", [TOK, D_OUT], _F32, kind="ExternalOutput")
        probe_d = None
    # DRAM views. x/We use a permuted contraction layout: partition p holds
    # rows 8p..8p+7 (j=0..7); both operands use the same permutation so the
    # contraction is unchanged. Every partition line is contiguous DRAM.
    xT2_v = xT2_d.rearrange("t (p j) n -> p t j n", p=P)       # [128,8,8,128]
    We_v = We_d.rearrange("e (p j) o -> p e j o", p=P)         # [128,8,8,1024]
    out_v = out_d.rearrange("(i p) o -> p i o", p=P)

    # Fewer DMA-completion semaphore lanes shrink the global wait set that
    # every loop exit/reset barrier (and its single-wait NoOp chunks) must
    # observe: 4 engine sems + N DMAHW lanes. Scoped to this build.
    import concourse.tile_sem_assignment as _tsa
    n_lanes = int(os.environ.get("MOE_DMA_LANES", "2"))
    saved_lanes = _tsa.NUM_HWDGE_SEMS
    _tsa.NUM_HWDGE_SEMS = n_lanes

    tc_cls = tile.TileContext if stock_tc else _ChunkedDrainTileContext
    try:
        return _build_nc_v2_body(nc, tc_cls, internal_io, reps, mmdt,
                                 xT2_d, We_d, be_d, Wg_d, bg_d, out_d,
                                 probe_d, xT2_v, We_v, out_v)
    finally:
        _tsa.NUM_HWDGE_SEMS = saved_lanes


def _build_nc_v2_body(nc, tc_cls, internal_io, reps, mmdt, xT2_d, We_d,
                      be_d, Wg_d, bg_d, out_d, probe_d, xT2_v, We_v, out_v):
    with tc_cls(nc) as tc, ExitStack() as ctx:
        singles = ctx.enter_context(tc.tile_pool(name="singles", bufs=1))
        # PSUM: pair pool 2x[128,1024] = 4 banks; ps_b holds gate logits
        # (2 banks) + exp-transpose staging (1 bank).
        ps_pair = ctx.enter_context(tc.tile_pool(name="ps_pair", bufs=2,
                                                 space="PSUM"))
        ps_b = ctx.enter_context(tc.tile_pool(name="ps_b", bufs=1,
                                              space="PSUM"))

        if internal_io:
            seed = singles.tile([P, D_OUT], _F32, tag="seed")
            nc.vector.memset(seed[:], 0.005)
            seedm = singles.tile([P, D_OUT], mmdt, tag="seedm")
            nc.scalar.copy(seedm[:], seed[:])

            def rep_src(n_rep, n_inner):
                s = seedm[:, :].opt()
                return bass.AP(tensor=s.tensor, offset=s.offset,
                               ap=[[s.ap[0][0], P], [0, n_rep], [1, n_inner]])

            for t in range(TT):
                nc.sync.dma_start(
                    xT2_d[t].rearrange("(p j) n -> p j n", p=P),
                    rep_src(KT, P))
            for e in range(E):
                nc.sync.dma_start(We_v[:, e], rep_src(KT, D_OUT))
            nc.sync.dma_start(be_d[:, :], seedm[0:E, :])
            nc.sync.dma_start(Wg_d.rearrange("(k p) e -> p k e", p=P),
                              seedm[:, 0:KT * E].rearrange(
                                  "p (k e) -> p k e", k=KT))
            nc.sync.dma_start(bg_d[:], seed[0, 0:E])

        # --- setup (outside the rep loop; compiled once, not per rep) ---
        ident8f = singles.tile([E, E], _F32, tag="id8f")
        make_identity(nc, ident8f)
        ident8 = singles.tile([E, E], mmdt, tag="id8")
        nc.scalar.copy(ident8[:], ident8f[:])

        # Wg in the same permuted-k layout (partition p = rows 8p..8p+7)
        wg_sb = singles.tile([P, KT, E], mmdt, tag="wg")
        nc.sync.dma_start(wg_sb[:], Wg_d.rearrange("(p j) e -> p j e", p=P))
        bg_col = singles.tile([E, 1], _F32, tag="bg")
        nc.sync.dma_start(bg_col[:], bg_d[:])
        be_sb = singles.tile([E, D_OUT], mmdt, tag="be")
        nc.sync.dma_start(be_sb[:], be_d[:, :])

        def _rep_body():
            # x for the gate in one contiguous load: [p, i, j, n]
            xg = singles.tile([P, TT, KT, P], mmdt, tag="xg")
            nc.sync.dma_start(xg[:], xT2_v)

            # --- gate: logits^T [E, tok] with E on partitions, tok=(i,n) ---
            pg = ps_b.tile([E, TT, P], _F32, tag="g")
            for j in range(KT):
                for h in range(2):
                    nc.tensor.matmul(
                        pg[:, h * (TT // 2):(h + 1) * (TT // 2), :],
                        wg_sb[:, j, :],
                        xg[:, h * (TT // 2):(h + 1) * (TT // 2), j, :],
                        start=(j == 0), stop=(j == KT - 1))
            # exp(logits + bg) in one ACT op (bias is a per-partition operand)
            expT = singles.tile([E, TOK], mmdt, tag="expT")
            nc.scalar.activation(expT[:], pg.rearrange("e a b -> e (a b)"),
                                 mybir.ActivationFunctionType.Exp,
                                 bias=bg_col[:])
            # transpose exp to token layout: [P, TT*E], col i*E+e
            ptr = ps_b.tile([P, TT * E], mmdt, tag="tr")
            for i in range(TT):
                nc.tensor.transpose(ptr[:, i * E:(i + 1) * E],
                                    expT[:, i * P:(i + 1) * P], ident8[:])
            exp_tok = singles.tile([P, TT, E], _F32, tag="exptok")
            nc.vector.tensor_copy(exp_tok.rearrange("p a b -> p (a b)"),
                                  ptr[:])
            s_tok = singles.tile([P, TT, 1], _F32, tag="stok")
            nc.vector.reduce_sum(s_tok[:], exp_tok[:],
                                 axis=mybir.AxisListType.X)
            r_tok = singles.tile([P, TT, 1], _F32, tag="rtok")
            nc.vector.reciprocal(r_tok[:], s_tok[:])

            # --- bias init (static): acc[i] = exp-tile^T @ be per tile ---
            acc = singles.tile([P, TT, D_OUT], _F32, tag="acc")
            for i in range(TT):
                pb = ps_pair.tile([P, D_OUT], _F32, tag="pair")
                for h in range(2):
                    nc.tensor.matmul(pb[:, h * FH:(h + 1) * FH],
                                     expT[:, i * P:(i + 1) * P],
                                     be_sb[:, h * FH:(h + 1) * FH],
                                     start=True, stop=True)
                nc.vector.tensor_copy(acc[:, i, :], pb[:])

            exp_flat = exp_tok.rearrange("p a b -> p (a b)")
            acc_flat = acc.rearrange("p a b -> p (a b)")

            # --- main loops: experts outer (weights staged once per e),
            # token tiles inner (x tile staged per iteration). All matmul
            # operands are static APs; only DMA/DVE use register offsets.
            with tc.For_i(0, E, 1) as e:
                westage = singles.tile([P, KT, D_OUT], mmdt, tag="westage")
                nc.sync.dma_start(westage[:], We_v[:, bass.ds(e, 1), :, :])
                with tc.For_i(0, TT, 1) as i:
                    xstage = singles.tile([P, KT, P], mmdt, tag="xstage")
                    nc.sync.dma_start(xstage[:],
                                      xT2_v[:, bass.ds(i, 1), :, :])
                    pm = ps_pair.tile([P, D_OUT], _F32, tag="pair")
                    for j in range(KT):
                        for h in range(2):
                            nc.tensor.matmul(
                                pm[:, h * FH:(h + 1) * FH], xstage[:, j, :],
                                westage[:, j, h * FH:(h + 1) * FH],
                                start=(j == 0), stop=(j == KT - 1),
                            )
                    acc_slice = acc_flat[:, bass.ds(i * D_OUT, D_OUT)]
                    nc.vector.scalar_tensor_tensor(
                        out=acc_slice, in0=pm[:],
                        scalar=exp_flat[:, bass.ds(i * E + e, 1)],
                        in1=acc_slice,
                        op0=mybir.AluOpType.mult, op1=mybir.AluOpType.add,
                    )

            # normalize: acc *= 1/sum(exp), broadcast over D_OUT
            r = r_tok[:, :, 0:1].opt()
            rb = bass.AP(tensor=r.tensor, offset=r.offset,
                         ap=[r.ap[0], r.ap[1], [0, D_OUT]])
            nc.vector.tensor_mul(acc[:], acc[:], rb)
            nc.sync.dma_start(out_v, acc[:])
            return acc

        for _ in range(reps):
            acc_last = _rep_body()

        if internal_io:
            nc.sync.dma_start(probe_d[:, :], acc_last[:, 0, 0:P])

    return nc


_NC_CACHE: dict = {}


def _get_nc(mode: str, reps: int = 1) -> bass.Bass:
    key = (mode, reps)
    if key not in _NC_CACHE:
        _NC_CACHE[key] = build_nc(mode, reps)
    return _NC_CACHE[key]


def make_in_maps(x, We, be, Wg, bg, mode: str = MODE):
    import ml_dtypes

    dt_np = ml_dtypes.bfloat16 if mode == "bf16" else np.float32
    We_c = np.ascontiguousarray(We, dtype=dt_np)
    be_c = np.ascontiguousarray(be, dtype=dt_np)
    Wg_c = np.ascontiguousarray(Wg, dtype=dt_np)
    bg_c = np.ascontiguousarray(bg, dtype=np.float32)
    in_maps = []
    for c in range(NCORES):
        xs = np.asarray(x[c * TOK:(c + 1) * TOK], dtype=dt_np)
        # token-tile-major transpose: xT2[i, k, n] = x[i*128 + n, k]
        xT2 = np.ascontiguousarray(
            xs.reshape(TT, P, D_IN).transpose(0, 2, 1))
        in_maps.append({
            "xT2": xT2,
            "We": We_c,
            "be": be_c,
            "Wg": Wg_c,
            "bg": bg_c,
        })
    return in_maps


def kernel(x, We, be, Wg, bg):
    key = ("v2", MODE)
    if key not in _NC_CACHE:
        _NC_CACHE[key] = build_nc_v2("bf16", reps=1)
    nc = _NC_CACHE[key]
    in_maps = make_in_maps(x, We, be, Wg, bg, "bf16")
    res = run_bass_kernel_spmd(nc, in_maps, list(range(NCORES)))
    out = np.concatenate([res.results[c]["out"] for c in range(NCORES)], axis=0)
    return out.astype(np.float32)



# revision 42
# speedup vs baseline: 15.6996x; 4.2540x over previous
"""Trainium2 Bass kernel for the dense MoE layer (nn_MoELayer_74371653698164).

Reference computation (fp32):
    gate  = softmax(x @ Wg + bg)                    # [N, E]
    out   = sum_e gate[:, e] * (x @ We[e] + be[e])  # [N, D_OUT]

Shipped kernel: build_nc_v2 (bf16, hardware-looped; see its docstring).
Data-parallel over tokens: each of the 8 cores gets N/8 = 1024 tokens and
the full expert/gate weights, no collectives. The dominant cost in this
harness scales with the STATIC instruction count of the program (per-call
NEFF compile/load, ~40 us/instruction), so the 64 (expert, token-tile)
GEMM iterations run in two nested tc.For_i hardware loops instead of
being unrolled: ~420 static instructions per rep instead of ~2300.

build_nc below is the earlier fully-unrolled f32r variant, kept for A/B.

Original baseline notes:
  - Data-parallel over tokens: each of the 8 cores gets N/8 = 1024 tokens and
    the full expert/gate weights. No collectives.
  - x is pre-transposed on the host (a weights-style layout change), so the
    contraction dim lands on partitions with a single DMA and no on-device
    transpose pass.
  - Softmax is factored: out = r * (sum_e exp_e * (x @ We[e] + be[e])) with
    r = 1 / sum_e exp_e. Logits are computed TRANSPOSED ([E, tok], E on
    partitions) so the bias add is a per-partition scalar op and exp is one
    activation over all tokens; logits here are ~N(0,1) so max-subtraction
    is unnecessary in fp32.
  - Per expert: stream We[e] from HBM, run 128x512 matmuls into a 2-bank
    [128,1024] PSUM pair, and fold the gate in with one DVE FMA per
    (expert, token-tile): acc = psum * exp[:, e] + acc. The bias term
    (exp @ be, one K=8 matmul pair per token tile) initializes acc.
  - Matmul operands are float32r: full fp32 data that the PE streams at
    1 cycle/row for N>=256 (vs 4 for strict fp32), near-fp32 precision.
  - Instruction count is minimized throughout (batched drains, paired PSUM
    banks, single big DMAs) — both dispatch overhead and real-HW sync cost
    scale with it.

kernel(**inputs) takes the FULL unsharded inputs and returns the FULL output.
"""
import os
from contextlib import ExitStack

import numpy as np

import bass_rust
import concourse.bass as bass
import concourse.mybir as mybir
import concourse.tile as tile
from concourse.bass_utils import run_bass_kernel_spmd
from concourse.masks import make_identity
from concourse.vector_clock import ScopedClock

# Problem shape (hardcoded per harness contract).
N_TOKENS, D_IN, D_OUT, E = 8192, 1024, 1024, 8
NCORES = 8
TOK = N_TOKENS // NCORES  # tokens per core
P = 128                   # partitions
KT = D_IN // P            # contraction tiles
TT = TOK // P             # token tiles per core
FH = 512                  # max fp32 matmul free dim (one PSUM bank)

# "f32r" (default): fp32 data, PE in float32r mode (fast, ~fp32 precision)
# "f32": strict fp32 matmuls (4x slower PE)
# "bf16": bf16 inputs (half DMA traffic, ~3e-3 rel err)
MODE = os.environ.get("MOE_KERNEL_MODE", "f32r")

_F32 = mybir.dt.float32
_F32R = mybir.dt.float32r
_BF16 = mybir.dt.bfloat16


class _ChunkedDrainTileContext(tile.TileContext):
    """TileContext adapted to a walrus that allows ONE sync wait per
    instruction ("Too many sync wait commands", CoreV3GenImpl setupSyncWait).

    Stock Tile attaches up to ~3 waits to an instruction (and the whole
    global-clock wait set to the tail drain). Every extra wait is hoisted
    onto a same-engine InstNoOp carrier emitted immediately before the
    instruction, so the engine's sequencer observes the sems in order.
    """

    _HOIST_WAITS = os.environ.get("MOE_HOIST_WAITS", "0") == "1"

    def __init__(self, *a, **kw):
        super().__init__(*a, **kw)
        self._last_by_engine = {}

    # Walrus setupSyncWait allows exactly ONE wait per instruction on this
    # toolchain — including NoOp/Drain (probed: 2 and 4 both rejected).
    _NOOP_WAITS = int(os.environ.get("MOE_NOOP_WAITS", "1"))

    def _add_instruction(self, inst):
        si = getattr(inst, "sync_info", None)
        cap = (self._NOOP_WAITS
               if isinstance(inst, (mybir.InstNoOp, mybir.InstDrain)) else 1)
        if si is not None and si.on_wait and len(si.on_wait) > cap:
            waits = list(si.on_wait)
            if cap > 1:
                head, waits = waits[:-cap], waits[-cap:]
                for g0 in range(0, len(head), cap):
                    g = head[g0:g0 + cap]
                    nop = mybir.InstNoOp(
                        name=self.nc.get_next_instruction_name(), ins=[],
                        outs=[])
                    nop.engine = inst.engine
                    nop.bass_nofuse = True
                    nop.sync_info = bass_rust.SyncInfo(on_wait=g, on_update=[])
                    super()._add_instruction(nop)
                inst.sync_info = bass_rust.SyncInfo(
                    on_wait=waits, on_update=list(si.on_update or []))
                self._last_by_engine[inst.engine] = inst
                super()._add_instruction(inst)
                return
            # Optionally park one extra wait on the immediately preceding
            # same-engine instruction when it carries no waits/updates of its
            # own: the wait just fires one slot earlier in the same stream.
            if self._HOIST_WAITS and len(waits) == 2:
                prev = self._last_by_engine.get(inst.engine)
                psi = getattr(prev, "sync_info", None) if prev is not None else None
                if prev is not None and (
                    psi is None or (not psi.on_wait and not psi.on_update)
                ):
                    prev.sync_info = bass_rust.SyncInfo(
                        on_wait=[waits[0]], on_update=[])
                    waits = waits[1:]
            for w in waits[:-1]:
                nop = mybir.InstNoOp(
                    name=self.nc.get_next_instruction_name(), ins=[], outs=[]
                )
                nop.engine = inst.engine
                nop.bass_nofuse = True
                nop.sync_info = bass_rust.SyncInfo(on_wait=[w], on_update=[])
                super()._add_instruction(nop)
            inst.sync_info = bass_rust.SyncInfo(
                on_wait=[waits[-1]], on_update=list(si.on_update or [])
            )
        self._last_by_engine[inst.engine] = inst
        super()._add_instruction(inst)

    def _drain_and_barrier(self, tick_clock, wait_clock):
        drain_inst = self.nc.sync.drain()
        wait_clock.add_sem_waits(
            drain_inst.ins, ScopedClock({None: tick_clock.global_clock})
        )
        si = drain_inst.ins.sync_info
        waits = list(si.on_wait or []) if si is not None else []
        if len(waits) > 1:
            drain_inst.ins.sync_info = bass_rust.SyncInfo(
                on_wait=waits[:1], on_update=list(si.on_update or [])
            )
            for w in waits[1:]:
                extra = self.nc.sync.drain()
                extra.ins.sync_info = bass_rust.SyncInfo(on_wait=[w], on_update=[])

        self.nc.all_engine_barrier()
        assert self.sems is not None
        popped = self.nc._tile_sem_poison_stack.pop()
        assert popped is self._sem_poison
        self.nc.clear_and_free_semaphores(list(self.sems.allocated().values()))
        self.nc.all_engine_barrier()


def build_nc(mode: str = MODE, reps: int = 1, internal_io: bool = False,
             n_experts: int = E, do_gate: bool = True, do_bias: bool = True,
             do_fma: bool = True, do_store: bool = True, do_mm: bool = True,
             we_loads: int | None = None, fh: int = FH,
             hw_loop: bool = False) -> bass.Bass:
    """Build the per-core Bass program.

    reps: repeat the compute body (timing harnesses amortize dispatch
    overhead); internal_io: inputs live in internal DRAM seeded on-device
    (timing without host transfers); remaining flags ablate stages.
    """
    mmdt = {"bf16": _BF16, "f32": _F32, "f32r": _F32R}[mode]

    nc = bass.Bass()
    kind_in = {} if internal_io else {"kind": "ExternalInput"}
    xT_d = nc.dram_tensor("xT", [D_IN, TOK], mmdt, **kind_in)
    We_d = nc.dram_tensor("We", [E, D_IN, D_OUT], mmdt, **kind_in)
    be_d = nc.dram_tensor("be", [E, D_OUT], mmdt, **kind_in)
    Wg_d = nc.dram_tensor("Wg", [D_IN, E], mmdt, **kind_in)
    bg_d = nc.dram_tensor("bg", [E], _F32, **kind_in)
    if internal_io:
        out_d = nc.dram_tensor("out", [TOK, D_OUT], _F32)
        probe_d = nc.dram_tensor("probe", [P, P], _F32, kind="ExternalOutput")
    else:
        out_d = nc.dram_tensor("out", [TOK, D_OUT], _F32, kind="ExternalOutput")
        probe_d = None

    with _ChunkedDrainTileContext(nc) as tc, ExitStack() as ctx:
        singles = ctx.enter_context(tc.tile_pool(name="singles", bufs=1))
        wepool = ctx.enter_context(tc.tile_pool(name="we", bufs=2))
        # PSUM budget: pair pool 2x[128,1024] = 4 banks; ps_b holds the gate
        # logits tile (2 banks) + the exp-transpose staging tile (1 bank).
        ps_pair = ctx.enter_context(tc.tile_pool(name="ps_pair", bufs=2,
                                                 space="PSUM"))
        ps_b = ctx.enter_context(tc.tile_pool(name="ps_b", bufs=1,
                                              space="PSUM"))

        if internal_io:
            # Seed internal inputs on-device with benign constants (values
            # are irrelevant to timing; softmax logits stay small/finite).
            seed = singles.tile([P, D_OUT], _F32, tag="seed")
            nc.vector.memset(seed[:], 0.005)
            if mmdt != _F32:
                seedm = singles.tile([P, D_OUT], mmdt, tag="seedm")
                nc.scalar.copy(seedm[:], seed[:])
            else:
                seedm = seed

            def rep_src(n_rep):
                s = seedm[:, :].opt()
                return bass.AP(tensor=s.tensor, offset=s.offset,
                               ap=[[s.ap[0][0], P], [0, n_rep], [1, D_OUT]])

            nc.sync.dma_start(xT_d.rearrange("(k p) n -> p k n", p=P),
                              rep_src(KT))
            for e in range(E):
                nc.sync.dma_start(We_d[e].rearrange("(k p) o -> p k o", p=P),
                                  rep_src(KT))
            nc.sync.dma_start(be_d[:, :], seedm[0:E, :])
            nc.sync.dma_start(Wg_d.rearrange("(k p) e -> p k e", p=P),
                              seedm[:, 0:KT * E].rearrange(
                                  "p (k e) -> p k e", k=KT))
            nc.sync.dma_start(bg_d[:], seed[0, 0:E])

        # small identity for the [E,tok] -> [tok,E] exp transpose
        ident8f = singles.tile([E, E], _F32, tag="id8f")
        make_identity(nc, ident8f)
        if mmdt != _F32:
            ident8 = singles.tile([E, E], mmdt, tag="id8")
            nc.scalar.copy(ident8[:], ident8f[:])
        else:
            ident8 = ident8f

        wg_sb = singles.tile([P, KT, E], mmdt, tag="wg")
        nc.sync.dma_start(wg_sb[:], Wg_d.rearrange("(k p) e -> p k e", p=P))
        bg_col = singles.tile([E, 1], _F32, tag="bg")
        nc.sync.dma_start(bg_col[:], bg_d[:])
        be_sb = singles.tile([E, D_OUT], mmdt, tag="be")
        nc.sync.dma_start(be_sb[:], be_d[:, :])

        def _rep_body():
            xT = singles.tile([P, KT, TOK], mmdt, tag="xT")
            nc.sync.dma_start(xT[:], xT_d.rearrange("(k p) n -> p k n", p=P))
            acc = singles.tile([P, TT, D_OUT], _F32, tag="acc")

            if do_gate:
                # logits^T [E, tok] in PSUM (E on partitions), bias add as a
                # per-partition scalar, exp over all tokens at once.
                pg = ps_b.tile([E, TOK], _F32, tag="g")
                for k in range(KT):
                    for h in range(TOK // FH):
                        nc.tensor.matmul(
                            pg[:, h * FH:(h + 1) * FH], wg_sb[:, k, :],
                            xT[:, k, h * FH:(h + 1) * FH],
                            start=(k == 0), stop=(k == KT - 1),
                        )
                ltT = singles.tile([E, TOK], _F32, tag="ltT")
                nc.vector.tensor_scalar_add(ltT[:], pg[:], bg_col[:])
                expT = singles.tile([E, TOK], mmdt, tag="expT")
                nc.scalar.activation(expT[:], ltT[:],
                                     mybir.ActivationFunctionType.Exp)
                # transpose exp to token layout: 8 blocks into one PSUM bank
                ptr = ps_b.tile([P, TT * E], mmdt, tag="tr")
                for i in range(TT):
                    nc.tensor.transpose(ptr[:, i * E:(i + 1) * E],
                                        expT[:, i * P:(i + 1) * P], ident8[:])
                exp_tok = singles.tile([P, TT, E], _F32, tag="exptok")
                nc.scalar.copy(exp_tok.rearrange("p a b -> p (a b)"), ptr[:])
                s_tok = singles.tile([P, TT, 1], _F32, tag="stok")
                nc.vector.reduce_sum(s_tok[:], exp_tok[:],
                                     axis=mybir.AxisListType.X)
                r_tok = singles.tile([P, TT, 1], _F32, tag="rtok")
                nc.vector.reciprocal(r_tok[:], s_tok[:])

            if do_bias:
                # acc init: (unnormalized) exp @ be
                for i in range(TT):
                    pb = ps_pair.tile([P, D_OUT], _F32, tag="pair")
                    for h in range(D_OUT // FH):
                        nc.tensor.matmul(
                            pb[:, h * FH:(h + 1) * FH],
                            expT[:, i * P:(i + 1) * P],
                            be_sb[:, h * FH:(h + 1) * FH],
                            start=True, stop=True,
                        )
                    nc.scalar.copy(acc[:, i, :], pb[:])

            # experts: acc += exp[:, e] * (x @ We[e])
            we = None
            for e in range(n_experts):
                if we_loads is None or e < we_loads:
                    we = wepool.tile([P, KT, D_OUT], mmdt, tag="we")
                    nc.sync.dma_start(
                        we[:], We_d[e].rearrange("(k p) o -> p k o", p=P))
                for i in range(TT if do_mm else 0):
                    isl = slice(i * P, (i + 1) * P)
                    pm = ps_pair.tile([P, D_OUT], _F32, tag="pair")
                    for k in range(KT):
                        for h in range(D_OUT // fh):
                            nc.tensor.matmul(
                                pm[:, h * fh:(h + 1) * fh], xT[:, k, isl],
                                we[:, k, h * fh:(h + 1) * fh],
                                start=(k == 0), stop=(k == KT - 1),
                            )
                    if do_fma:
                        nc.vector.scalar_tensor_tensor(
                            out=acc[:, i, :], in0=pm[:],
                            scalar=exp_tok[:, i, e:e + 1], in1=acc[:, i, :],
                            op0=mybir.AluOpType.mult, op1=mybir.AluOpType.add,
                        )

            if do_gate and do_fma:
                # normalize: acc *= 1/sum(exp), broadcast over D_OUT
                r = r_tok[:, :, 0:1].opt()
                rb = bass.AP(tensor=r.tensor, offset=r.offset,
                             ap=[r.ap[0], r.ap[1], [0, D_OUT]])
                nc.vector.tensor_mul(acc[:], acc[:], rb)

            if do_store:
                nc.sync.dma_start(out_d.rearrange("(i p) o -> p i o", p=P),
                                  acc[:])
            return acc

        if hw_loop:
            with tc.For_i(0, reps, 1):
                acc = _rep_body()
        else:
            for _ in range(reps):
                acc = _rep_body()

        if internal_io:
            nc.sync.dma_start(probe_d[:, :], acc[:, 0, 0:P])

    return nc


def build_nc_v2(mode: str = "bf16", reps: int = 1, internal_io: bool = False,
                stock_tc: bool = False) -> bass.Bass:
    """Instruction-lean build: the 64 (expert, token-tile) GEMM iterations
    run in two nested tc.For_i hardware loops, so the static program is a
    few hundred instructions instead of a few thousand.

    Constraints discovered on this toolchain that shape the design:
      - ldweights (matmul stationary operand) cannot take register offsets,
        so the stationary x token-tile is staged into a fixed SBUF tile by
        DMA (which can).
      - register-offset DMA must source from DRAM (SBUF needs bacc), so
        x is staged from a token-tile-major DRAM copy (xT2) and exp^T from
        a DRAM scratch dump.
      - matmul PSUM output is capped at one bank (512 f32), so every
        matmul writes F=512 halves.
      - register allocation has no spilling and registers leak across
        loop constructs, so dynamic APs are kept to a handful per rep
        (all matmul operands are static APs).
      - walrus allows ONE sync wait per compute instruction
        (_ChunkedDrainTileContext hoists extras onto NoOp carriers).

    Per-expert weights are staged from HBM once per expert (16 MB/rep
    total, same traffic as a full preload); x tiles are restaged per
    (expert, tile) at 32 KiB each (2 MB/rep, contiguous 2 KiB lines).
    """
    assert mode == "bf16"
    mmdt = _BF16

    nc = bass.Bass()
    kind_in = {} if internal_io else {"kind": "ExternalInput"}
    # x shard, token-tile-major: xT2[i, k, n] = x[i*128 + n, k]
    xT2_d = nc.dram_tensor("xT2", [TT, D_IN, P], mmdt, **kind_in)
    We_d = nc.dram_tensor("We", [E, D_IN, D_OUT], mmdt, **kind_in)
    be_d = nc.dram_tensor("be", [E, D_OUT], mmdt, **kind_in)
    Wg_d = nc.dram_tensor("Wg", [D_IN, E], mmdt, **kind_in)
    bg_d = nc.dram_tensor("bg", [E], _F32, **kind_in)
    if internal_io:
        out_d = nc.dram_tensor("out", [TOK, D_OUT], _F32)
        probe_d = nc.dram_tensor("probe", [P, P], _F32, kind="ExternalOutput")
    else:
        out_d = nc.dram_tensor("out", [TOK, D_OUT], _F32, kind="ExternalOutput")
        probe_d = None
    # DRAM views. x/We use a permuted contraction layout: partition p holds
    # rows 8p..8p+7 (j=0..7); both operands use the same permutation so the
    # contraction is unchanged. Every partition line is contiguous DRAM.
    xT2_v = xT2_d.rearrange("t (p j) n -> p t j n", p=P)       # [128,8,8,128]
    We_v = We_d.rearrange("e (p j) o -> p e j o", p=P)         # [128,8,8,1024]
    out_v = out_d.rearrange("(i p) o -> p i o", p=P)

    # Fewer DMA-completion semaphore lanes shrink the global wait set that
    # every loop exit/reset barrier (and its single-wait NoOp chunks) must
    # observe: 4 engine sems + N DMAHW lanes. Scoped to this build.
    import concourse.tile_sem_assignment as _tsa
    n_lanes = int(os.environ.get("MOE_DMA_LANES", "2"))
    saved_lanes = _tsa.NUM_HWDGE_SEMS
    _tsa.NUM_HWDGE_SEMS = n_lanes

    tc_cls = tile.TileContext if stock_tc else _ChunkedDrainTileContext
    try:
        return _build_nc_v2_body(nc, tc_cls, internal_io, reps, mmdt,
                                 xT2_d, We_d, be_d, Wg_d, bg_d, out_d,
                                 probe_d, xT2_v, We_v, out_v)
    finally:
        _tsa.NUM_HWDGE_SEMS = saved_lanes


def _build_nc_v2_body(nc, tc_cls, internal_io, reps, mmdt, xT2_d, We_d,
                      be_d, Wg_d, bg_d, out_d, probe_d, xT2_v, We_v, out_v):
    with tc_cls(nc) as tc, ExitStack() as ctx:
        singles = ctx.enter_context(tc.tile_pool(name="singles", bufs=1))
        # PSUM: pair pool 2x[128,1024] = 4 banks; ps_b holds gate logits
        # (2 banks) + exp-transpose staging (1 bank).
        ps_pair = ctx.enter_context(tc.tile_pool(name="ps_pair", bufs=2,
                                                 space="PSUM"))
        ps_b = ctx.enter_context(tc.tile_pool(name="ps_b", bufs=1,
                                              space="PSUM"))

        if internal_io:
            seed = singles.tile([P, D_OUT], _F32, tag="seed")
            nc.vector.memset(seed[:], 0.005)
            seedm = singles.tile([P, D_OUT], mmdt, tag="seedm")
            nc.scalar.copy(seedm[:], seed[:])

            def rep_src(n_rep, n_inner):
                s = seedm[:, :].opt()
                return bass.AP(tensor=s.tensor, offset=s.offset,
                               ap=[[s.ap[0][0], P], [0, n_rep], [1, n_inner]])

            for t in range(TT):
                nc.sync.dma_start(
                    xT2_d[t].rearrange("(p j) n -> p j n", p=P),
                    rep_src(KT, P))
            for e in range(E):
                nc.sync.dma_start(We_v[:, e], rep_src(KT, D_OUT))
            nc.sync.dma_start(be_d[:, :], seedm[0:E, :])
            nc.sync.dma_start(Wg_d.rearrange("(k p) e -> p k e", p=P),
                              seedm[:, 0:KT * E].rearrange(
                                  "p (k e) -> p k e", k=KT))
            nc.sync.dma_start(bg_d[:], seed[0, 0:E])

        # --- setup (outside the rep loop; compiled once, not per rep) ---
        # exp^T lives in a 16-partition tile (DMA transpose needs src
        # partitions % 16); rows E..2E stay zero forever.
        expT = singles.tile([2 * E, TOK], mmdt, tag="expT")
        nc.vector.memset(expT[:], 0.0)

        # Wg in the same permuted-k layout (partition p = rows 8p..8p+7)
        wg_sb = singles.tile([P, KT, E], mmdt, tag="wg")
        nc.sync.dma_start(wg_sb[:], Wg_d.rearrange("(p j) e -> p j e", p=P))
        bg_col = singles.tile([E, 1], _F32, tag="bg")
        nc.sync.dma_start(bg_col[:], bg_d[:])
        be_sb = singles.tile([E, D_OUT], mmdt, tag="be")
        nc.sync.dma_start(be_sb[:], be_d[:, :])

        def _rep_body():
            # x for the gate in one contiguous load: [p, i, j, n]
            xg = singles.tile([P, TT, KT, P], mmdt, tag="xg")
            nc.sync.dma_start(xg[:], xT2_v)

            # --- gate: logits^T [E, tok] with E on partitions, tok=(i,n) ---
            pg = ps_b.tile([E, TT, P], _F32, tag="g")
            for j in range(KT):
                for h in range(2):
                    nc.tensor.matmul(
                        pg[:, h * (TT // 2):(h + 1) * (TT // 2), :],
                        wg_sb[:, j, :],
                        xg[:, h * (TT // 2):(h + 1) * (TT // 2), j, :],
                        start=(j == 0), stop=(j == KT - 1))
            # exp(logits + bg) in one ACT op (bias is a per-partition operand)
            nc.scalar.activation(expT[0:E, :], pg.rearrange("e a b -> e (a b)"),
                                 mybir.ActivationFunctionType.Exp,
                                 bias=bg_col[:])
            # blocked transpose to token layout in ONE DMA:
            # exp_tok[p, i, e] = expT[e, i*128+p]  (cols E..2E are zeros)
            exp_tok = singles.tile([P, TT, 2 * E], mmdt, tag="exptok")
            nc.sync.dma_start(exp_tok[:],
                              expT.rearrange("e (i p) -> e i p", p=P),
                              transpose=True)
            s_tok = singles.tile([P, TT, 1], _F32, tag="stok")
            nc.vector.reduce_sum(s_tok[:], exp_tok[:, :, 0:E],
                                 axis=mybir.AxisListType.X)
            r_tok = singles.tile([P, TT, 1], _F32, tag="rtok")
            nc.vector.reciprocal(r_tok[:], s_tok[:])

            # --- bias init (static): acc[i] = exp-tile^T @ be per tile ---
            acc = singles.tile([P, TT, D_OUT], _F32, tag="acc")
            for i in range(TT):
                pb = ps_pair.tile([P, D_OUT], _F32, tag="pair")
                for h in range(2):
                    nc.tensor.matmul(pb[:, h * FH:(h + 1) * FH],
                                     expT[0:E, i * P:(i + 1) * P],
                                     be_sb[:, h * FH:(h + 1) * FH],
                                     start=True, stop=True)
                nc.vector.tensor_copy(acc[:, i, :], pb[:])

            exp_flat = exp_tok.rearrange("p a b -> p (a b)")
            acc_flat = acc.rearrange("p a b -> p (a b)")

            # --- main loops: experts outer (weights staged once per e),
            # token tiles inner (x tile staged per iteration). All matmul
            # operands are static APs; only DMA/DVE use register offsets.
            with tc.For_i(0, E, 1) as e:
                westage = singles.tile([P, KT, D_OUT], mmdt, tag="westage")
                nc.sync.dma_start(westage[:], We_v[:, bass.ds(e, 1), :, :])
                with tc.For_i(0, TT, 1) as i:
                    xstage = singles.tile([P, KT, P], mmdt, tag="xstage")
                    nc.sync.dma_start(xstage[:],
                                      xT2_v[:, bass.ds(i, 1), :, :])
                    pm = ps_pair.tile([P, D_OUT], _F32, tag="pair")
                    for j in range(KT):
                        for h in range(2):
                            nc.tensor.matmul(
                                pm[:, h * FH:(h + 1) * FH], xstage[:, j, :],
                                westage[:, j, h * FH:(h + 1) * FH],
                                start=(j == 0), stop=(j == KT - 1),
                            )
                    acc_slice = acc_flat[:, bass.ds(i * D_OUT, D_OUT)]
                    nc.vector.scalar_tensor_tensor(
                        out=acc_slice, in0=pm[:],
                        scalar=exp_flat[:, bass.ds(i * (2 * E) + e, 1)],
                        in1=acc_slice,
                        op0=mybir.AluOpType.mult, op1=mybir.AluOpType.add,
                    )

            # normalize: acc *= 1/sum(exp), broadcast over D_OUT
            r = r_tok[:, :, 0:1].opt()
            rb = bass.AP(tensor=r.tensor, offset=r.offset,
                         ap=[r.ap[0], r.ap[1], [0, D_OUT]])
            nc.vector.tensor_mul(acc[:], acc[:], rb)
            nc.sync.dma_start(out_v, acc[:])
            return acc

        for _ in range(reps):
            acc_last = _rep_body()

        if internal_io:
            nc.sync.dma_start(probe_d[:, :], acc_last[:, 0, 0:P])

    return nc


_NC_CACHE: dict = {}


def _get_nc(mode: str, reps: int = 1) -> bass.Bass:
    key = (mode, reps)
    if key not in _NC_CACHE:
        _NC_CACHE[key] = build_nc(mode, reps)
    return _NC_CACHE[key]


def make_in_maps(x, We, be, Wg, bg, mode: str = MODE):
    import ml_dtypes

    dt_np = ml_dtypes.bfloat16 if mode == "bf16" else np.float32
    We_c = np.ascontiguousarray(We, dtype=dt_np)
    be_c = np.ascontiguousarray(be, dtype=dt_np)
    Wg_c = np.ascontiguousarray(Wg, dtype=dt_np)
    bg_c = np.ascontiguousarray(bg, dtype=np.float32)
    in_maps = []
    for c in range(NCORES):
        xs = np.asarray(x[c * TOK:(c + 1) * TOK], dtype=dt_np)
        # token-tile-major transpose: xT2[i, k, n] = x[i*128 + n, k]
        xT2 = np.ascontiguousarray(
            xs.reshape(TT, P, D_IN).transpose(0, 2, 1))
        in_maps.append({
            "xT2": xT2,
            "We": We_c,
            "be": be_c,
            "Wg": Wg_c,
            "bg": bg_c,
        })
    return in_maps


def kernel(x, We, be, Wg, bg):
    key = ("v2", MODE)
    if key not in _NC_CACHE:
        _NC_CACHE[key] = build_nc_v2("bf16", reps=1)
    nc = _NC_CACHE[key]
    in_maps = make_in_maps(x, We, be, Wg, bg, "bf16")
    res = run_bass_kernel_spmd(nc, in_maps, list(range(NCORES)))
    out = np.concatenate([res.results[c]["out"] for c in range(NCORES)], axis=0)
    return out.astype(np.float32)

